# revision 8
# baseline (speedup 1.0000x reference)
"""Trainium2 Bass kernel for nn_MultiHeadAttention_88716844467031 (B=1, NQ=NK=2048,
D=1024, H=16, DK=DV=64, M=64 memory slots, returns (ln_out, masked_scores)).

Sharding over 8 NeuronCores: head-group x query-quarter hybrid.
  core c: head group g = c % 2   (8 heads: 8g..8g+7)
          query quarter i = c // 2  (512 queries: [512i, 512i+512))
k/v projections are computed per nk-quarter and re-united with a 4-rank
AllGather (groups {g, g+2, g+4, g+6}). Pair {2i, 2i+1} covers all 16 heads of
quarter i; a 2-rank AllGather of pv^T re-unites the heads for the output
projection; both pair members compute fc_o + layernorm for the whole quarter
and the host keeps each core's half.

Scores are computed transposed (S^T: [nk, nq]) so softmax sums (via an
interleaved ones-column in the value matrix) and the P@V matmul need no
on-chip transposes; aw / (pa - inf*mask) are pre-transposed on the host,
r_att is emitted transposed and re-transposed on the host. 1/sqrt(dk) is
folded into Wq on the host. The q/k/S path runs fp32r by default (bf16 via
KERNEL_S_DTYPE=bf16); the p@v / fc_o path runs bf16.
"""

import os
from contextlib import ExitStack

import numpy as np
import ml_dtypes

import concourse.bass as bass
import concourse.bacc as bacc
import concourse.mybir as mybir
import concourse.tile as tile
from concourse.bass_utils import run_bass_kernel_spmd

F32 = mybir.dt.float32
F32R = mybir.dt.float32r
BF16 = mybir.dt.bfloat16
AF = mybir.ActivationFunctionType
OP = mybir.AluOpType

B, NQ, NK, D, H, DK, DV, M = 1, 2048, 2048, 1024, 16, 64, 64, 64
EPS = 1e-5
N_CORES = 8
HC = 8            # heads per core
NQC = 512         # queries per core (quarter)
NKE = 2176        # 2048 keys + 64 memory slots + 64 pad
NKT = 17          # nk tiles of 128 (tile 16: 64 memory + 64 pad)
VW = 65           # per-head width in v_sb (64 v cols + 1 ones col)

S_BF16 = os.environ.get("KERNEL_S_DTYPE", "f32r") == "bf16"
SDT = BF16 if S_BF16 else F32R
SNP = ml_dtypes.bfloat16 if S_BF16 else np.float32

LAST_RESULTS = None
_CACHED_NC = None


def _build():
    nc = bacc.Bacc("TRN2", target_bir_lowering=False, debug=False,
                   num_devices=N_CORES)

    def din(name, shape, dt):
        return nc.dram_tensor(name, list(shape), dt, kind="ExternalInput")

    qT_in = din("qT_in", [D, NQC], SDT)            # queries^T quarter slice
    keysT_q = din("keysT_q", [D, NQC], SDT)        # keys^T nk-quarter slice
    valsT1_q = din("valsT1_q", [D + 1, NQC], F32R)  # values^T + ones row, quarter
    wqT = din("wqT", [D, HC * DK], SDT)            # (Wq/8)^T, this head group
    bq_c = din("bq_c", [128, 4], F32)
    wkT = din("wkT", [D, HC * DK], SDT)
    bk_c = din("bk_c", [128, 4], F32)
    wvT1 = din("wvT1", [D + 1, HC * DV], F32R)     # Wv^T + bv row
    woT = din("woT", [H * DV, D], BF16)            # Wo^T (full)
    awT_c = din("awT_c", [NKE, NQC], F32)
    paMT_c = din("paMT_c", [NKE, NQC], F32)
    m_k_c = din("m_k_c", [HC, DK, M], SDT)
    m_v_c = din("m_v_c", [HC, M, DV], BF16)
    qb_c = din("qb_c", [NQC, D], F32)              # quarter queries rows + bo
    gamma_bc = din("gamma_bc", [128, D], F32)
    beta_bc = din("beta_bc", [128, D], F32)

    r_out = nc.dram_tensor("r_attT", [HC, NK + M, NQC], F32, kind="ExternalOutput")
    y_out = nc.dram_tensor("y_out", [NQC, D], F32, kind="ExternalOutput")

    with tile.TileContext(nc) as tc, ExitStack() as ctx:
        kT_pool = ctx.enter_context(tc.tile_pool(name="kTp", bufs=1))
        v_pool = ctx.enter_context(tc.tile_pool(name="vp", bufs=1))
        qT_pool = ctx.enter_context(tc.tile_pool(name="qTp", bufs=1))
        aw_pool = ctx.enter_context(tc.tile_pool(name="awp", bufs=1))
        const_pool = ctx.enter_context(tc.tile_pool(name="constp", bufs=1))
        dram = ctx.enter_context(tc.tile_pool(name="dramp", bufs=1, space="DRAM"))

        kT_sb = kT_pool.tile([128, 4 * NKE], SDT)      # 4 part-groups x 2176
        v_sb = v_pool.tile([128, NKT * 8 * VW], BF16)  # 17 nk-tiles x 8h x 65
        qT_sb = qT_pool.tile([128, 4 * NQC], SDT)
        awT_sb = aw_pool.tile([128, NKT * NQC], F32)
        paMT_sb = aw_pool.tile([128, NKT * NQC], F32)

        bq_sb = const_pool.tile([128, 4], F32)
        bk_sb = const_pool.tile([128, 4], F32)

        nc.sync.dma_start(bq_sb[:], bq_c[:])
        nc.sync.dma_start(bk_sb[:], bk_c[:])
        nc.sync.dma_start(
            awT_sb[:].rearrange("p (t n) -> p t n", n=NQC),
            awT_c[:].rearrange("(t p) n -> p t n", p=128))
        nc.sync.dma_start(
            paMT_sb[:].rearrange("p (t n) -> p t n", n=NQC),
            paMT_c[:].rearrange("(t p) n -> p t n", p=128))

        # memory-slot / pad init for the extended key & value matrices
        for grp in range(4):
            nc.vector.memset(
                kT_sb[:, grp * NKE + 2112:(grp + 1) * NKE].bitcast(F32)
                if not S_BF16 else kT_sb[:, grp * NKE + 2112:(grp + 1) * NKE],
                0.0)
        nc.vector.memset(v_sb[64:128, 16 * 8 * VW:17 * 8 * VW], 0.0)
        v4 = v_sb[:].rearrange("p (t h w) -> p t h w", h=8, w=VW)
        v3 = v_sb[:].rearrange("p (th w) -> p th w", w=VW)
        nc.vector.memset(v3[:, :, 64:65], 1.0)         # interleaved ones cols
        for h in range(HC):
            po, grp = 64 * (h % 2), h // 2
            nc.sync.dma_start(
                kT_sb[po:po + 64, grp * NKE + 2048:grp * NKE + 2112], m_k_c[h])
            nc.sync.dma_start(
                v_sb[0:64, 16 * 8 * VW + VW * h:16 * 8 * VW + VW * h + 64],
                m_v_c[h])

        # ------------------------- projections -------------------------
        ib_k = dram.tile([4 * 128, NQC], SDT)
        ob_k = dram.tile([16 * 128, NQC], SDT)
        ib_v = dram.tile([4 * 128, NQC], BF16)
        ob_v = dram.tile([16 * 128, NQC], BF16)
        AG4 = [[0, 2, 4, 6], [1, 3, 5, 7]]

        with (
            tc.tile_pool(name="pw", bufs=1) as pw,
            tc.tile_pool(name="prhs", bufs=6) as prhs,
            tc.tile_pool(name="pstage", bufs=1) as pstage,
            tc.tile_pool(name="psum_p", bufs=4, space="PSUM") as psum_p,
        ):
            # qT = (Wq/8) @ queries_slice^T -> [512 hdk, 512 nq]
            w_sb = pw.tile([128, 8 * 512], SDT, tag="w", name="wq_sb")
            qin_sb = pw.tile([128, 8 * NQC], SDT, tag="qin")
            nc.sync.dma_start(
                w_sb[:].rearrange("p (k m) -> p k m", m=512),
                wqT[:].rearrange("(k p) m -> p k m", p=128))
            nc.sync.dma_start(
                qin_sb[:].rearrange("p (k n) -> p k n", n=NQC),
                qT_in[:].rearrange("(k p) n -> p k n", p=128))
            for m in range(4):
                ps = psum_p.tile([128, NQC], F32, tag="ps")
                for k in range(8):
                    nc.tensor.matmul(
                        ps[:],
                        w_sb[:, 512 * k + 128 * m:512 * k + 128 * (m + 1)],
                        qin_sb[:, NQC * k:NQC * (k + 1)],
                        start=(k == 0), stop=(k == 7))
                nc.vector.tensor_scalar_add(
                    qT_sb[:, NQC * m:NQC * (m + 1)], ps[:], bq_sb[:, m:m + 1])

            # kT quarter = Wk @ keys_q^T -> [512 hdk, 512 nk]; AG4 -> full kT
            wk_sb = pw.tile([128, 8 * 512], SDT, tag="w", name="wk_sb")
            nc.sync.dma_start(
                wk_sb[:].rearrange("p (k m) -> p k m", m=512),
                wkT[:].rearrange("(k p) m -> p k m", p=128))
            kq_sb = pstage.tile([128, 4 * NQC], SDT)
            kt = [prhs.tile([128, NQC], SDT, tag="krhs", name=f"kt{j}")
                  for j in range(8)]
            for k in range(8):
                nc.sync.dma_start(kt[k][:], keysT_q[128 * k:128 * (k + 1), :])
            for m in range(4):
                ps = psum_p.tile([128, NQC], F32, tag="ps")
                for k in range(8):
                    nc.tensor.matmul(
                        ps[:],
                        wk_sb[:, 512 * k + 128 * m:512 * k + 128 * (m + 1)],
                        kt[k][:],
                        start=(k == 0), stop=(k == 7))
                nc.vector.tensor_scalar_add(
                    kq_sb[:, NQC * m:NQC * (m + 1)], ps[:], bk_sb[:, m:m + 1])
                nc.sync.dma_start(ib_k[128 * m:128 * (m + 1), :],
                                  kq_sb[:, NQC * m:NQC * (m + 1)])
            nc.gpsimd.collective_compute(
                "AllGather", OP.bypass, replica_groups=AG4,
                ins=[ib_k[:].opt()], outs=[ob_k[:].opt()])
            for b in range(4):
                for m in range(4):
                    nc.sync.dma_start(
                        kT_sb[:, NKE * m + 512 * b:NKE * m + 512 * (b + 1)],
                        ob_k[512 * b + 128 * m:512 * b + 128 * (m + 1), :])

            # v quarter = values_q @ Wv^T + bv -> [512 nk, 512 hdv]; AG4
            wv_sb = pw.tile([128, 8 * 512], F32R, tag="w", name="wv_sb")
            wv1_sb = pw.tile([1, 512], F32R)
            nc.sync.dma_start(
                wv_sb[:].rearrange("p (k m) -> p k m", m=512),
                wvT1[0:D].rearrange("(k p) m -> p k m", p=128))
            nc.sync.dma_start(wv1_sb[:], wvT1[D:D + 1, :])
            vq_sb = pstage.tile([128, 4 * NQC], BF16)
            for t in range(4):
                vt = [prhs.tile([128, 128], F32R, tag="vrhs", name=f"vt{t}_{j}")
                      for j in range(8)]
                vt1 = prhs.tile([1, 128], F32R, tag="vrhs1")
                for k in range(8):
                    nc.sync.dma_start(vt[k][:], valsT1_q[128 * k:128 * (k + 1),
                                                         128 * t:128 * (t + 1)])
                nc.sync.dma_start(vt1[:], valsT1_q[D:D + 1, 128 * t:128 * (t + 1)])
                ps = psum_p.tile([128, 512], F32, tag="ps")
                for k in range(8):
                    nc.tensor.matmul(ps[:], vt[k][:],
                                     wv_sb[:, 512 * k:512 * (k + 1)],
                                     start=(k == 0), stop=False)
                nc.tensor.matmul(ps[:], vt1[:], wv1_sb[:], start=False, stop=True)
                nc.scalar.copy(vq_sb[:, 512 * t:512 * (t + 1)], ps[:])
                nc.sync.dma_start(ib_v[128 * t:128 * (t + 1), :],
                                  vq_sb[:, 512 * t:512 * (t + 1)])
            nc.gpsimd.collective_compute(
                "AllGather", OP.bypass, replica_groups=AG4,
                ins=[ib_v[:].opt()], outs=[ob_v[:].opt()])
            for t in range(16):
                vrow = prhs.tile([128, 512], BF16, tag="vrow", name=f"vrow{t}")
                nc.sync.dma_start(vrow[:], ob_v[128 * t:128 * (t + 1), :])
                nc.scalar.copy(
                    v4[:, t, :, 0:64],
                    vrow[:].rearrange("p (h n) -> p h n", n=64))

        # ------------------------- attention -------------------------
        with (
            tc.tile_pool(name="pvbuf", bufs=1) as pvbuf,
            tc.tile_pool(name="work", bufs=4) as work,
            tc.tile_pool(name="sstagep", bufs=2) as sstagep,
            tc.tile_pool(name="psum_s", bufs=4, space="PSUM") as psum_s,
            tc.tile_pool(name="psum_pv", bufs=2, space="PSUM") as psum_pv,
            tc.tile_pool(name="rbp", bufs=2) as rbp,
        ):
            pvU_sb = pvbuf.tile([64, HC * NQC], BF16)
            sums_sb = pvbuf.tile([HC, NQC], F32)
            rs_sb = pvbuf.tile([HC, NQC], BF16)
            pvN_sb = pvbuf.tile([64, HC * NQC], BF16)
            sums_dram = dram.tile([HC, NQC], F32)
            rs_dram = dram.tile([HC, NQC], BF16)

            for h in range(HC):
                po, grp = 64 * (h % 2), h // 2
                pv_ps = psum_pv.tile([VW, NQC], F32, tag="pv")
                for t in range(NKT):
                    s_ps = psum_s.tile([128, NQC], F32, tag="s")
                    nc.tensor.matmul(
                        s_ps[:],
                        kT_sb[po:po + 64,
                              NKE * grp + 128 * t:NKE * grp + 128 * (t + 1)],
                        qT_sb[po:po + 64, NQC * grp:NQC * (grp + 1)],
                        start=True, stop=True)
                    tmp = work.tile([128, NQC], F32, tag="tmp")
                    nc.vector.tensor_mul(tmp[:], s_ps[:],
                                         awT_sb[:, NQC * t:NQC * (t + 1)])
                    r = work.tile([128, NQC], F32, tag="r")
                    if t % 3 == 2:
                        nc.gpsimd.tensor_add(r[:], tmp[:],
                                             paMT_sb[:, NQC * t:NQC * (t + 1)])
                    else:
                        nc.vector.tensor_add(r[:], tmp[:],
                                             paMT_sb[:, NQC * t:NQC * (t + 1)])
                    if t < 16:
                        nc.sync.dma_start(r_out[h, 128 * t:128 * (t + 1), :], r[:])
                    else:
                        nc.sync.dma_start(r_out[h, 2048:2112, :], r[0:64, :])
                    e = work.tile([128, NQC], BF16, tag="e")
                    nc.scalar.activation(e[:], r[:], AF.Exp)
                    nc.tensor.matmul(
                        pv_ps[:],
                        v_sb[:, 8 * VW * t + VW * h:8 * VW * t + VW * (h + 1)],
                        e[:], start=(t == 0), stop=(t == NKT - 1))
                nc.scalar.copy(pvU_sb[0:64, NQC * h:NQC * (h + 1)], pv_ps[0:64, :])
                sstage = sstagep.tile([VW, NQC], F32, tag="sst")
                nc.scalar.copy(sstage[64:65, :], pv_ps[64:65, :])
                nc.sync.dma_start(sums_dram[h:h + 1, :], sstage[64:65, :])

            nc.sync.dma_start(sums_sb[:], sums_dram[:])
            with nc.allow_low_precision(reason="softmax denom recip in bf16"):
                nc.vector.reciprocal(rs_sb[:], sums_sb[:])
            nc.sync.dma_start(rs_dram[:], rs_sb[:])
            for h in range(HC):
                rb = rbp.tile([64, NQC], BF16, tag="rb")
                sl = rs_dram[h:h + 1, :]
                src = bass.AP(sl.tensor, sl.offset, [[0, 64], [1, NQC]])
                nc.sync.dma_start(rb[:], src)
                nc.vector.tensor_mul(pvN_sb[0:64, NQC * h:NQC * (h + 1)],
                                     pvU_sb[0:64, NQC * h:NQC * (h + 1)], rb[:])

            # pair exchange of pv^T (bf16, 0.5 MB per rank)
            ib = dram.tile([HC * DV, NQC], BF16)
            ob = dram.tile([H * DV, NQC], BF16)
            for h in range(HC):
                nc.sync.dma_start(ib[64 * h:64 * (h + 1), :],
                                  pvN_sb[0:64, NQC * h:NQC * (h + 1)])
            nc.gpsimd.collective_compute(
                "AllGather", OP.bypass,
                replica_groups=[[0, 1], [2, 3], [4, 5], [6, 7]],
                ins=[ib[:].opt()], outs=[ob[:].opt()])

        # --------------------- fc_o + residual + LN ---------------------
        with (
            tc.tile_pool(name="late", bufs=1) as late,
            tc.tile_pool(name="fo", bufs=4) as fo,
            tc.tile_pool(name="fw", bufs=2) as fw,
            tc.tile_pool(name="psum_o", bufs=8, space="PSUM") as psum_o,
            tc.tile_pool(name="ln", bufs=2) as ln,
            tc.tile_pool(name="stat", bufs=10) as stat,
        ):
            gamma_sb = late.tile([128, D], F32)
            eps_sb = late.tile([128, 1], F32)
            nc.vector.memset(eps_sb[:], EPS)
            beta_sb = late.tile([128, D], F32)
            qb_sb = late.tile([128, 4, D], F32)
            nc.sync.dma_start(gamma_sb[:], gamma_bc[:])
            nc.sync.dma_start(beta_sb[:], beta_bc[:])
            nc.sync.dma_start(qb_sb[:],
                              qb_c[:].rearrange("(m p) d -> p m d", p=128))

            y_ps = [psum_o.tile([128, 512], F32, tag="yp", name=f"y_ps{j}")
                    for j in range(8)]
            for k in range(8):
                wo_k = fw.tile([128, D], BF16, tag="wo")
                nc.sync.dma_start(wo_k[:], woT[128 * k:128 * (k + 1), :])
                for mi in range(4):
                    lo = fo.tile([128, 128], BF16, tag="lhs")
                    nc.sync.dma_start(lo[:], ob[128 * k:128 * (k + 1),
                                                128 * mi:128 * (mi + 1)])
                    for ni in range(2):
                        nc.tensor.matmul(
                            y_ps[2 * mi + ni][:], lo[:],
                            wo_k[:, 512 * ni:512 * (ni + 1)],
                            start=(k == 0), stop=(k == 7))
            for mi in range(4):
                x = ln.tile([128, D], F32, tag="x")
                for ni in range(2):
                    nc.vector.tensor_add(
                        x[:, 512 * ni:512 * (ni + 1)], y_ps[2 * mi + ni][:],
                        qb_sb[:, mi, 512 * ni:512 * (ni + 1)])
                sum_c = stat.tile([128, 1], F32, tag="sum")
                nc.vector.tensor_reduce(sum_c[:], x[:],
                                        axis=mybir.AxisListType.X, op=OP.add)
                mu = stat.tile([128, 1], F32, tag="mu")
                nc.vector.tensor_scalar_mul(mu[:], sum_c[:], 1.0 / D)
                xc = ln.tile([128, D], F32, tag="xc")
                nc.vector.tensor_scalar_sub(xc[:], x[:], mu[:])
                sq = ln.tile([128, D], F32, tag="sq")
                ssq = stat.tile([128, 1], F32, tag="ssq")
                nc.scalar.activation(sq[:], xc[:], AF.Square, accum_out=ssq[:])
                var = stat.tile([128, 1], F32, tag="var")
                nc.vector.tensor_scalar_mul(var[:], ssq[:], 1.0 / D)
                sd = stat.tile([128, 1], F32, tag="sd")
                nc.scalar.activation(sd[:], var[:], AF.Sqrt, bias=eps_sb[:])
                rstd = stat.tile([128, 1], F32, tag="rstd")
                nc.vector.reciprocal(rstd[:], sd[:])
                y1 = ln.tile([128, D], F32, tag="y1")
                nc.vector.scalar_tensor_tensor(y1[:], xc[:], rstd[:],
                                               gamma_sb[:],
                                               op0=OP.mult, op1=OP.mult)
                yo = ln.tile([128, D], F32, tag="yo")
                nc.vector.tensor_add(yo[:], y1[:], beta_sb[:])
                nc.sync.dma_start(y_out[128 * mi:128 * (mi + 1), :], yo[:])

    nc.compile()
    return nc


def _host_prep(queries, keys, values, attention_weights, prev_att,
               attention_mask, Wq, bq, Wk, bk, Wv, bv, Wo, bo, m_k, m_v,
               gamma, beta):
    f32 = np.float32
    q2 = np.asarray(queries, f32)[0]
    qT = np.ascontiguousarray(q2.T)
    keys_T = np.ascontiguousarray(np.asarray(keys, f32)[0].T)
    vT = np.asarray(values, f32)[0].T
    valsT1 = np.concatenate([vT, np.ones((1, NK), f32)], axis=0)
    awT = np.asarray(attention_weights, f32)[0].T
    paM = np.where(np.asarray(attention_mask)[0], -np.inf,
                   np.asarray(prev_att, f32)[0])
    paMT = paM.T
    WoT_bf = np.ascontiguousarray(np.asarray(Wo, f32).T).astype(ml_dtypes.bfloat16)
    qb = q2 + np.asarray(bo, f32)[None, :]
    gamma_bc = np.ascontiguousarray(
        np.broadcast_to(np.asarray(gamma, f32), (128, D)))
    beta_bc = np.ascontiguousarray(
        np.broadcast_to(np.asarray(beta, f32), (128, D)))
    Wq, bq, Wk, bk, Wv, bv = (np.asarray(a, f32) for a in (Wq, bq, Wk, bk, Wv, bv))
    m_k, m_v = np.asarray(m_k, f32), np.asarray(m_v, f32)

    shared_g = {}
    for g in range(2):
        hsl = slice(512 * g, 512 * (g + 1))
        WvT1_c = np.empty((D + 1, HC * DV), f32)
        WvT1_c[0:D] = Wv[hsl].T
        WvT1_c[D] = bv[hsl]
        shared_g[g] = {
            "wqT": (np.ascontiguousarray(Wq[hsl].T) / 8.0).astype(SNP),
            "bq_c": np.ascontiguousarray((bq[hsl] / 8.0).reshape(4, 128).T),
            "wkT": np.ascontiguousarray(Wk[hsl].T).astype(SNP),
            "bk_c": np.ascontiguousarray(bk[hsl].reshape(4, 128).T),
            "wvT1": WvT1_c,
            "m_k_c": np.ascontiguousarray(m_k[0, 8 * g:8 * (g + 1)]).astype(SNP),
            "m_v_c": np.ascontiguousarray(
                m_v[0, 8 * g:8 * (g + 1)]).astype(ml_dtypes.bfloat16),
        }
    shared_i = {}
    for i in range(4):
        qsl = slice(512 * i, 512 * (i + 1))
        awT_ext = np.empty((NKE, NQC), f32)
        awT_ext[0:NK] = awT[:, qsl]
        awT_ext[NK:NK + M] = 1.0
        awT_ext[NK + M:] = 0.0
        paMT_ext = np.empty((NKE, NQC), f32)
        paMT_ext[0:NK] = paMT[:, qsl]
        paMT_ext[NK:NK + M] = 0.0
        paMT_ext[NK + M:] = -np.inf
        shared_i[i] = {
            "qT_in": np.ascontiguousarray(qT[:, qsl]).astype(SNP),
            "keysT_q": np.ascontiguousarray(keys_T[:, qsl]).astype(SNP),
            "valsT1_q": np.ascontiguousarray(valsT1[:, qsl]),
            "awT_c": awT_ext,
            "paMT_c": paMT_ext,
            "qb_c": np.ascontiguousarray(qb[qsl]),
        }

    in_maps = []
    for c in range(N_CORES):
        g, i = c % 2, c // 2
        im = {
            "woT": WoT_bf,
            "gamma_bc": gamma_bc,
            "beta_bc": beta_bc,
        }
        im.update(shared_g[g])
        im.update(shared_i[i])
        in_maps.append(im)
    return in_maps


def kernel(**inputs):
    global LAST_RESULTS, _CACHED_NC
    if _CACHED_NC is None:
        _CACHED_NC = _build()
    nc = _CACHED_NC

    in_maps = _host_prep(**inputs)
    res = run_bass_kernel_spmd(nc, in_maps, core_ids=list(range(N_CORES)))
    LAST_RESULTS = res

    out = np.empty((NQ, D), np.float32)
    r_att = np.empty((H, NQ, NK + M), np.float32)
    for c in range(N_CORES):
        g, i = c % 2, c // 2
        out[512 * i + 256 * g:512 * i + 256 * (g + 1)] = \
            res.results[c]["y_out"][256 * g:256 * (g + 1)]
        r_att[8 * g:8 * (g + 1), 512 * i:512 * (i + 1), :] = \
            res.results[c]["r_attT"].transpose(0, 2, 1)
    return out[None], r_att[None]
